# revision 28
# baseline (speedup 1.0000x reference)
"""Low-rank layer y = x @ (U diag(s) V^T)^T on 8 TRN2 NeuronCores.

Factored as two thin matmuls per core (data-parallel over batch, 1 batch/core):
  stage 1: t[r, n]  = sum_i (V*s)[i, r] * x[n, i]   (contraction i on partitions)
  stage 2: y[n, o]  = sum_r t[r, n] * U[o, r]       (contraction r on partitions)

Host prep: fold s into V, transpose x per core so the contraction dim lands on
SBUF partitions, pre-tile weights into partition-major layouts, cast matmul
operands to bf16 (PSUM accumulates fp32; y returned as fp32).
"""

import numpy as np
import ml_dtypes

import concourse.bass as bass
import concourse.mybir as mybir
import concourse.tile as tile
from concourse.tile import ScopedClock
from concourse.bass_utils import run_bass_kernel_spmd

P = 128
B = 8
TOKENS = 2048
D_IN = 4096
D_OUT = 4096
R = 256
I_CHUNKS = D_IN // P  # 32
R_HALVES = R // P  # 2
N_TILE = 512
T_TILES = TOKENS // N_TILE  # 4
TOK_BLKS = TOKENS // P  # 16
O_TILES = D_OUT // N_TILE  # 8


def _patched_drain_and_barrier(self, tick_clock, wait_clock):
    # This walrus build's CoreV3 CTRL lowering accepts at most one sync-wait
    # on the TileContext-exit SP Drain; split the global-clock waits across a
    # chain of SP nops (one wait each) emitted just before the drain.
    nc = self.nc
    lead = nc.sync.nop(nofuse=True, hint="tile_drain_wait_split")
    wait_clock.add_sem_waits(lead.ins, ScopedClock({None: tick_clock.global_clock}))
    si = lead.ins.sync_info
    waits = list(si.on_wait or [])
    if len(waits) > 1:
        si.on_wait = waits[:1]
        for w in waits[1:]:
            extra = nc.sync.nop(nofuse=True, hint="tile_drain_wait_split")
            esi = extra.ins.sync_info
            if esi is None:
                extra.ins.sync_info = mybir.SyncInfo(on_wait=[w], on_update=[])
            else:
                esi.on_wait = [w]
    nc.sync.drain()
    nc.all_engine_barrier()
    assert self.sems is not None
    popped = nc._tile_sem_poison_stack.pop()
    assert popped is self._sem_poison
    nc.clear_and_free_semaphores(list(self.sems.allocated().values()))
    nc.all_engine_barrier()


def _install_drain_patch():
    if not getattr(tile.TileContext, "_drain_patch_installed", False):
        tile.TileContext._drain_and_barrier = _patched_drain_and_barrier
        tile.TileContext._drain_patch_installed = True


def _legalize_waits(nc):
    # This walrus build accepts at most one sync-wait per instruction.
    # Hoist extra waits onto same-engine nops inserted just before the
    # offending instruction (same engine queue -> identical blocking).
    for fn in nc.m.functions:
        for bb in fn.blocks:
            new_list = []
            for inst in list(bb.instructions):
                si = inst.sync_info
                waits = list(si.on_wait) if si and si.on_wait else []
                if len(waits) > 1:
                    for w in waits[:-1]:
                        nop = nc.engines[inst.engine].nop(
                            nofuse=True, hint="wait_split"
                        )
                        cur = nc.cur_bb.bb.instructions
                        assert cur[-1] is nop.ins
                        cur.pop()
                        nsi = nop.ins.sync_info
                        if nsi is None:
                            nop.ins.sync_info = mybir.SyncInfo(
                                on_wait=[w], on_update=[]
                            )
                        else:
                            nsi.on_wait = [w]
                        new_list.append(nop.ins)
                    si.on_wait = [waits[-1]]
                new_list.append(inst)
            bb.instructions[:] = new_list


# vs weight-load groups: tiny first group so the first matmul's weight wait is
# ~64KB, then bulk groups streamed just-in-time during stage 1.
VS_SIZES = [1, 7, 8, 8, 8]
VS_STARTS = [0, 1, 8, 16, 24]
VS_GROUPS = len(VS_SIZES)
X_SPLIT = 4  # first few x chunks arrive in quarters for a fast PE ramp


def _build(iodt=mybir.dt.bfloat16):
    f32 = mybir.dt.float32
    nc = bass.Bass()
    xT_d = nc.declare_dram_parameter("xT", [D_IN, TOKENS], iodt, isOutput=False)
    vs_d = [
        nc.declare_dram_parameter(f"vs{g}", [P, VS_SIZES[g], R], iodt, isOutput=False)
        for g in range(VS_GROUPS)
    ]
    ut_d = nc.declare_dram_parameter("ut", [R_HALVES, P, D_OUT], iodt, isOutput=False)
    y_d = nc.declare_dram_parameter("y", [TOKENS, D_OUT], iodt, isOutput=True)

    with tile.TileContext(nc) as tc:
        with (
            tc.tile_pool(name="consts", bufs=1) as consts,
            tc.tile_pool(name="xp", bufs=8) as xp,
            tc.tile_pool(name="t2p", bufs=1) as t2p,
            tc.tile_pool(name="yp", bufs=3) as yp,
            tc.tile_pool(name="psum", bufs=8, space="PSUM") as psum,
        ):
            # PSUM->SBUF copies round-robin over both PSUM-capable engines so
            # no single engine's copy chain gates PSUM-bank reuse by the
            # matmuls (GPSIMD cannot read PSUM).
            copy_engines = [nc.vector.tensor_copy, nc.scalar.copy]

            vs_sb = [
                consts.tile([P, VS_SIZES[g], R], iodt, name=f"vs{g}")
                for g in range(VS_GROUPS)
            ]
            ut_sb = [
                consts.tile([P, D_OUT], iodt, name=f"ut{h}") for h in range(R_HALVES)
            ]
            nc.sync.dma_start(out=vs_sb[0][:], in_=vs_d[0][:])

            # Weight DMA issue points: each group lands just before its first
            # consumer; ut streams in under stage 1's tail DMA slack.
            wload = {0: 1, 2: 2, 10: 3, 18: 4}
            uload = {22: 0, 26: 1}

            # stage 1: accumulate t[r, n] over the 32 i-chunks
            psum_t = [
                [psum.tile([P, N_TILE], f32, tag="ps", name="ps_t") for _ in range(T_TILES)]
                for _ in range(R_HALVES)
            ]
            for c in range(I_CHUNKS):
                xt = xp.tile([P, TOKENS], iodt, tag="xt", name="xt")
                if c < 2:
                    # quarter-granularity loads so the first matmuls' data
                    # waits are ~128KB, not a full 512KB chunk
                    q = TOKENS // X_SPLIT
                    for s in range(X_SPLIT):
                        nc.sync.dma_start(
                            out=xt[:, s * q : (s + 1) * q],
                            in_=xT_d[c * P : (c + 1) * P, s * q : (s + 1) * q],
                        )
                else:
                    nc.sync.dma_start(out=xt[:], in_=xT_d[c * P : (c + 1) * P, :])
                if c in wload:
                    g = wload[c]
                    nc.sync.dma_start(out=vs_sb[g][:], in_=vs_d[g][:])
                elif c in uload:
                    h = uload[c]
                    nc.sync.dma_start(out=ut_sb[h][:], in_=ut_d[h])
                g = max(i for i in range(VS_GROUPS) if VS_STARTS[i] <= c)
                for h in range(R_HALVES):
                    lhsT = vs_sb[g][:, c - VS_STARTS[g], h * P : (h + 1) * P]
                    for tt in range(T_TILES):
                        nc.tensor.matmul(
                            psum_t[h][tt],
                            lhsT,
                            xt[:, tt * N_TILE : (tt + 1) * N_TILE],
                            start=(c == 0),
                            stop=(c == I_CHUNKS - 1),
                        )

            # t back to SBUF (stage-2 stationary operand must live in SBUF);
            # tt-major order so stage 2's first token blocks unblock first.
            t2_sb = t2p.tile([P, R_HALVES, TOKENS], iodt)
            for k, (tt, h) in enumerate(
                (tt, h) for tt in range(T_TILES) for h in range(R_HALVES)
            ):
                copy_engines[k % 2](
                    out=t2_sb[:, h, tt * N_TILE : (tt + 1) * N_TILE],
                    in_=psum_t[h][tt],
                )

            # stage 2: y[n, o] accumulated over the 2 r-halves
            for tb in range(TOK_BLKS):
                y_sb = yp.tile([P, D_OUT], iodt, tag="yt", name="yt")
                psum_y = [
                    psum.tile([P, N_TILE], f32, tag="ps", name="ps_y")
                    for _ in range(O_TILES)
                ]
                for h in range(R_HALVES):
                    lhsT2 = t2_sb[:, h, tb * P : (tb + 1) * P]
                    for ot in range(O_TILES):
                        nc.tensor.matmul(
                            psum_y[ot],
                            lhsT2,
                            ut_sb[h][:, ot * N_TILE : (ot + 1) * N_TILE],
                            start=(h == 0),
                            stop=(h == R_HALVES - 1),
                        )
                half = D_OUT // 2
                for ot in range(O_TILES):
                    copy_engines[ot % 2](
                        out=y_sb[:, ot * N_TILE : (ot + 1) * N_TILE],
                        in_=psum_y[ot],
                    )
                    if ot == O_TILES // 2 - 1:
                        # first half streams out while the second half copies
                        nc.sync.dma_start(
                            out=y_d[tb * P : (tb + 1) * P, 0:half],
                            in_=y_sb[:, 0:half],
                        )
                nc.sync.dma_start(
                    out=y_d[tb * P : (tb + 1) * P, half:D_OUT],
                    in_=y_sb[:, half:D_OUT],
                )

    _legalize_waits(nc)
    return nc


_CACHED = {}


def kernel(x, u_approx, s_approx, v_approx, _trace=False):
    _install_drain_patch()
    bf16 = ml_dtypes.bfloat16

    vp = (v_approx.astype(np.float32) * s_approx.astype(np.float32)[None, :])
    vc = vp.reshape(I_CHUNKS, P, R)  # [chunk, partition, r]
    vs_host = [
        np.ascontiguousarray(
            vc[VS_STARTS[g] : VS_STARTS[g] + VS_SIZES[g]].transpose(1, 0, 2)
        ).astype(bf16)
        for g in range(VS_GROUPS)
    ]
    ut_host = np.ascontiguousarray(
        np.ascontiguousarray(u_approx.T).reshape(R_HALVES, P, D_OUT)
    ).astype(bf16)
    xT = [np.ascontiguousarray(x[b].T).astype(bf16) for b in range(B)]
    in_maps = [
        {"xT": xT[b], "ut": ut_host}
        | {f"vs{g}": vs_host[g] for g in range(VS_GROUPS)}
        for b in range(B)
    ]

    if "nc" not in _CACHED:
        _CACHED["nc"] = _build()
    res = run_bass_kernel_spmd(_CACHED["nc"], in_maps, list(range(B)), trace=_trace)
    y = np.stack(
        [np.asarray(res.results[b]["y"]).astype(np.float32) for b in range(B)]
    )
    if _trace:
        kernel.last_exec_time_ns = res.exec_time_ns
    return y



# revision 30
# speedup vs baseline: 1.0658x; 1.0658x over previous
"""Low-rank layer y = x @ (U diag(s) V^T)^T on 8 TRN2 NeuronCores.

Factored as two thin matmuls per core (data-parallel over batch, 1 batch/core):
  stage 1: t[r, n]  = sum_i (V*s)[i, r] * x[n, i]   (contraction i on partitions)
  stage 2: y[n, o]  = sum_r t[r, n] * U[o, r]       (contraction r on partitions)

Host prep: fold s into V, transpose x per core so the contraction dim lands on
SBUF partitions, pre-tile weights into partition-major layouts, cast matmul
operands to bf16 (PSUM accumulates fp32; y returned as fp32).
"""

import numpy as np
import ml_dtypes

import concourse.bass as bass
import concourse.mybir as mybir
import concourse.tile as tile
from concourse.tile import ScopedClock
from concourse.bass_utils import run_bass_kernel_spmd

P = 128
B = 8
TOKENS = 2048
D_IN = 4096
D_OUT = 4096
R = 256
I_CHUNKS = D_IN // P  # 32
R_HALVES = R // P  # 2
N_TILE = 512
T_TILES = TOKENS // N_TILE  # 4
TOK_BLKS = TOKENS // P  # 16
O_TILES = D_OUT // N_TILE  # 8


def _patched_drain_and_barrier(self, tick_clock, wait_clock):
    # This walrus build's CoreV3 CTRL lowering accepts at most one sync-wait
    # on the TileContext-exit SP Drain; split the global-clock waits across a
    # chain of SP nops (one wait each) emitted just before the drain.
    nc = self.nc
    lead = nc.sync.nop(nofuse=True, hint="tile_drain_wait_split")
    wait_clock.add_sem_waits(lead.ins, ScopedClock({None: tick_clock.global_clock}))
    si = lead.ins.sync_info
    waits = list(si.on_wait or [])
    if len(waits) > 1:
        si.on_wait = waits[:1]
        for w in waits[1:]:
            extra = nc.sync.nop(nofuse=True, hint="tile_drain_wait_split")
            esi = extra.ins.sync_info
            if esi is None:
                extra.ins.sync_info = mybir.SyncInfo(on_wait=[w], on_update=[])
            else:
                esi.on_wait = [w]
    nc.sync.drain()
    nc.all_engine_barrier()
    assert self.sems is not None
    popped = nc._tile_sem_poison_stack.pop()
    assert popped is self._sem_poison
    nc.clear_and_free_semaphores(list(self.sems.allocated().values()))
    nc.all_engine_barrier()


def _install_drain_patch():
    if not getattr(tile.TileContext, "_drain_patch_installed", False):
        tile.TileContext._drain_and_barrier = _patched_drain_and_barrier
        tile.TileContext._drain_patch_installed = True


def _legalize_waits(nc):
    # This walrus build accepts at most one sync-wait per instruction.
    # Hoist extra waits onto same-engine nops inserted just before the
    # offending instruction (same engine queue -> identical blocking).
    for fn in nc.m.functions:
        for bb in fn.blocks:
            new_list = []
            for inst in list(bb.instructions):
                si = inst.sync_info
                waits = list(si.on_wait) if si and si.on_wait else []
                if len(waits) > 1:
                    for w in waits[:-1]:
                        nop = nc.engines[inst.engine].nop(
                            nofuse=True, hint="wait_split"
                        )
                        cur = nc.cur_bb.bb.instructions
                        assert cur[-1] is nop.ins
                        cur.pop()
                        nsi = nop.ins.sync_info
                        if nsi is None:
                            nop.ins.sync_info = mybir.SyncInfo(
                                on_wait=[w], on_update=[]
                            )
                        else:
                            nsi.on_wait = [w]
                        new_list.append(nop.ins)
                    si.on_wait = [waits[-1]]
                new_list.append(inst)
            bb.instructions[:] = new_list


# vs weight-load groups: tiny first group so the first matmul's weight wait is
# ~64KB, then bulk groups streamed just-in-time during stage 1.
VS_SIZES = [1, 7, 8, 8, 8]
VS_STARTS = [0, 1, 8, 16, 24]
VS_GROUPS = len(VS_SIZES)
X_SPLIT = 4  # first few x chunks arrive in quarters for a fast PE ramp


def _build(iodt=mybir.dt.bfloat16):
    f32 = mybir.dt.float32
    nc = bass.Bass()
    xT_d = nc.declare_dram_parameter("xT", [D_IN, TOKENS], iodt, isOutput=False)
    vs_d = [
        nc.declare_dram_parameter(f"vs{g}", [P, VS_SIZES[g], R], iodt, isOutput=False)
        for g in range(VS_GROUPS)
    ]
    ut_d = nc.declare_dram_parameter("ut", [R_HALVES, P, D_OUT], iodt, isOutput=False)
    y_d = nc.declare_dram_parameter("y", [TOKENS, D_OUT], iodt, isOutput=True)

    with tile.TileContext(nc) as tc:
        with (
            tc.tile_pool(name="consts", bufs=1) as consts,
            tc.tile_pool(name="xp", bufs=10) as xp,
            tc.tile_pool(name="t2p", bufs=1) as t2p,
            tc.tile_pool(name="yp", bufs=3) as yp,
            tc.tile_pool(name="psum", bufs=8, space="PSUM") as psum,
        ):
            # PSUM->SBUF copies round-robin over both PSUM-capable engines so
            # no single engine's copy chain gates PSUM-bank reuse by the
            # matmuls (GPSIMD cannot read PSUM).
            copy_engines = [nc.vector.tensor_copy, nc.scalar.copy]

            vs_sb = [
                consts.tile([P, VS_SIZES[g], R], iodt, name=f"vs{g}")
                for g in range(VS_GROUPS)
            ]
            ut_sb = [
                consts.tile([P, D_OUT], iodt, name=f"ut{h}") for h in range(R_HALVES)
            ]
            nc.sync.dma_start(out=vs_sb[0][:], in_=vs_d[0][:])

            # Weight DMA issue points: each group lands just before its first
            # consumer; ut streams in under stage 1's tail DMA slack.
            wload = {0: 1, 2: 2, 10: 3, 18: 4}
            uload = {22: 0, 26: 1}

            # stage 1: accumulate t[r, n] over the 32 i-chunks
            psum_t = [
                [psum.tile([P, N_TILE], f32, tag="ps", name="ps_t") for _ in range(T_TILES)]
                for _ in range(R_HALVES)
            ]
            for c in range(I_CHUNKS):
                xt = xp.tile([P, TOKENS], iodt, tag="xt", name="xt")
                # split chunk loads (quarters early, halves after) so matmul
                # data waits are fine-grained and more DMAs pipeline in flight
                nsplit = X_SPLIT if c < 2 else 2
                q = TOKENS // nsplit
                for s in range(nsplit):
                    nc.sync.dma_start(
                        out=xt[:, s * q : (s + 1) * q],
                        in_=xT_d[c * P : (c + 1) * P, s * q : (s + 1) * q],
                    )
                if c in wload:
                    g = wload[c]
                    nc.sync.dma_start(out=vs_sb[g][:], in_=vs_d[g][:])
                elif c in uload:
                    h = uload[c]
                    nc.sync.dma_start(out=ut_sb[h][:], in_=ut_d[h])
                g = max(i for i in range(VS_GROUPS) if VS_STARTS[i] <= c)
                for h in range(R_HALVES):
                    lhsT = vs_sb[g][:, c - VS_STARTS[g], h * P : (h + 1) * P]
                    for tt in range(T_TILES):
                        nc.tensor.matmul(
                            psum_t[h][tt],
                            lhsT,
                            xt[:, tt * N_TILE : (tt + 1) * N_TILE],
                            start=(c == 0),
                            stop=(c == I_CHUNKS - 1),
                        )

            # t back to SBUF (stage-2 stationary operand must live in SBUF);
            # tt-major order so stage 2's first token blocks unblock first.
            t2_sb = t2p.tile([P, R_HALVES, TOKENS], iodt)
            for k, (tt, h) in enumerate(
                (tt, h) for tt in range(T_TILES) for h in range(R_HALVES)
            ):
                copy_engines[k % 2](
                    out=t2_sb[:, h, tt * N_TILE : (tt + 1) * N_TILE],
                    in_=psum_t[h][tt],
                )

            # stage 2: y[n, o] accumulated over the 2 r-halves
            for tb in range(TOK_BLKS):
                y_sb = yp.tile([P, D_OUT], iodt, tag="yt", name="yt")
                psum_y = [
                    psum.tile([P, N_TILE], f32, tag="ps", name="ps_y")
                    for _ in range(O_TILES)
                ]
                for h in range(R_HALVES):
                    lhsT2 = t2_sb[:, h, tb * P : (tb + 1) * P]
                    for ot in range(O_TILES):
                        nc.tensor.matmul(
                            psum_y[ot],
                            lhsT2,
                            ut_sb[h][:, ot * N_TILE : (ot + 1) * N_TILE],
                            start=(h == 0),
                            stop=(h == R_HALVES - 1),
                        )
                half = D_OUT // 2
                for ot in range(O_TILES):
                    copy_engines[ot % 2](
                        out=y_sb[:, ot * N_TILE : (ot + 1) * N_TILE],
                        in_=psum_y[ot],
                    )
                    if ot == O_TILES // 2 - 1:
                        # first half streams out while the second half copies
                        nc.sync.dma_start(
                            out=y_d[tb * P : (tb + 1) * P, 0:half],
                            in_=y_sb[:, 0:half],
                        )
                nc.sync.dma_start(
                    out=y_d[tb * P : (tb + 1) * P, half:D_OUT],
                    in_=y_sb[:, half:D_OUT],
                )

    _legalize_waits(nc)
    return nc


_CACHED = {}


def kernel(x, u_approx, s_approx, v_approx, _trace=False):
    _install_drain_patch()
    bf16 = ml_dtypes.bfloat16

    vp = (v_approx.astype(np.float32) * s_approx.astype(np.float32)[None, :])
    vc = vp.reshape(I_CHUNKS, P, R)  # [chunk, partition, r]
    vs_host = [
        np.ascontiguousarray(
            vc[VS_STARTS[g] : VS_STARTS[g] + VS_SIZES[g]].transpose(1, 0, 2)
        ).astype(bf16)
        for g in range(VS_GROUPS)
    ]
    ut_host = np.ascontiguousarray(
        np.ascontiguousarray(u_approx.T).reshape(R_HALVES, P, D_OUT)
    ).astype(bf16)
    xT = [np.ascontiguousarray(x[b].T).astype(bf16) for b in range(B)]
    in_maps = [
        {"xT": xT[b], "ut": ut_host}
        | {f"vs{g}": vs_host[g] for g in range(VS_GROUPS)}
        for b in range(B)
    ]

    if "nc" not in _CACHED:
        _CACHED["nc"] = _build()
    res = run_bass_kernel_spmd(_CACHED["nc"], in_maps, list(range(B)), trace=_trace)
    y = np.stack(
        [np.asarray(res.results[b]["y"]).astype(np.float32) for b in range(B)]
    )
    if _trace:
        kernel.last_exec_time_ns = res.exec_time_ns
    return y

